# revision 1
# baseline (speedup 1.0000x reference)
"""Trainium2 Bass kernel for nn_DifferentialAttention (sparse attention).

Reference computation (per batch element b):
    Q = x @ Wq + bq ; K = x @ Wk + bk ; V = x @ Wv + bv        [S, KD]
    scores  = Q @ K^T                                          [S, S]
    weights = softmax(scores, axis=-1)
    mask    = weights > mean(weights, axis=-1, keepdims=True)
    out     = (weights * mask) @ V                             [S, KD]

Key identities used:
  * mean(softmax row) == 1/S exactly, so the mask is  w_ij > 1/S.
  * With c_i = log(den_i / S)  (den_i = sum_j exp(s_ij)):
        w_ij * mask_ij = (den_i/S) * g_ij / den_i,
        g_ij = exp(s_ij - c_i) * [s_ij - c_i > 0]
    and  out_i = (1/S) * sum_j g_ij V_j   -- the denominator cancels.
  * g is computed from u = s - c in ONE scalar-engine exp plus ONE fused
    DVE op:  g = (e_u > 1) * e_u   (constant threshold!).
  * c_i is folded into the score matmul as an extra contraction row
    (lhsT = [K^T; ones], rhs = [Q^T; -c_row]), so no broadcast pass.

Sharding: 8 cores = (batch b in 0..3) x (query-row half h in 0..1).
Each core computes out[b, h*2048:(h+1)*2048, :].  The host feeds each
core x[b]^T with columns rotated so the core's own rows come first;
row order of K/V is softmax-invariant.
"""

import os
import sys

for _p in ("/opt/trn_rl_repo", "/opt/pypackages"):
    if _p not in sys.path and os.path.isdir(_p):
        sys.path.append(_p)

import numpy as np

import concourse.bass as bass
import concourse.tile as tile
from concourse import bacc, mybir

F32 = mybir.dt.float32
F32R = mybir.dt.float32r
EXP = mybir.ActivationFunctionType.Exp
LN = mybir.ActivationFunctionType.Ln
ADD = mybir.AluOpType.add
SUB = mybir.AluOpType.subtract
MULT = mybir.AluOpType.mult
IS_GT = mybir.AluOpType.is_gt

B, S, D, KD = 4, 4096, 256, 64
NCORES = 8
HALF = S // 2            # query rows per core (2048)
NCHUNK = HALF // 128     # 16 i-chunks of 128 rows
NJC = S // 128           # 32 j-chunks of 128 keys
LN_S = float(np.log(S))
# Global shift applied inside the round-1 exp so the fp32 denominators stay
# within the scalar engine's Ln range (scores reach +-65 on N(0,1) inputs).
M_SHIFT = 30.0


def build_program(repeat: int = 1, phase: int = 4) -> bass.Bass:
    """repeat>1 builds the same kernel body N times back-to-back (timing aid)."""
    nc = bacc.Bacc("TRN2", target_bir_lowering=False, debug=False)

    xT_d = nc.dram_tensor("xT", [D, S], F32, kind="ExternalInput")
    wq_d = nc.dram_tensor("Wq", [D, KD], F32, kind="ExternalInput")
    wk_d = nc.dram_tensor("Wk", [D, KD], F32, kind="ExternalInput")
    wv_d = nc.dram_tensor("Wv", [D, KD], F32, kind="ExternalInput")
    bqc_d = nc.dram_tensor("bq_col", [KD, 1], F32, kind="ExternalInput")
    bkc_d = nc.dram_tensor("bk_col", [KD, 1], F32, kind="ExternalInput")
    bvr_d = nc.dram_tensor("bv_row", [1, KD], F32, kind="ExternalInput")
    ones_d = nc.dram_tensor("ones", [1, S], F32R, kind="ExternalInput")
    ones32_d = nc.dram_tensor("ones32", [1, 128], F32, kind="ExternalInput")
    onescol_d = nc.dram_tensor("ones_col", [128, 1], F32R, kind="ExternalInput")
    out_d = nc.dram_tensor("out", [HALF, KD], F32, kind="ExternalOutput")

    ident_d = nc.inline_tensor(np.eye(128, dtype=np.float32), name="ident")

    with tile.TileContext(nc) as tc:
        with (
            tc.tile_pool(name="const", bufs=1) as cst,
            tc.tile_pool(name="vpool", bufs=1) as vpool,
            tc.tile_pool(name="scratch", bufs=3) as scr,
            tc.tile_pool(name="work", bufs=5) as work,
        ):
            for _rep in range(repeat):
                # ---- load inputs -------------------------------------------------
                xt = []
                for dc in range(2):
                    t = cst.tile([128, S], F32, tag=f"xt{dc}")
                    # split the 2MB load so Q/K projections can start earlier
                    nc.sync.dma_start(t[:, 0:HALF], xT_d[dc * 128:(dc + 1) * 128, 0:HALF])
                    nc.sync.dma_start(t[:, HALF:S], xT_d[dc * 128:(dc + 1) * 128, HALF:S])
                    xt.append(t)
                wq, wk, wv = [], [], []
                for dc in range(2):
                    for (name, lst, dram) in (("wq", wq, wq_d), ("wk", wk, wk_d),
                                              ("wv", wv, wv_d)):
                        t = cst.tile([128, KD], F32, tag=f"{name}{dc}")
                        nc.sync.dma_start(t[:], dram[dc * 128:(dc + 1) * 128, :])
                        lst.append(t)
                bq_c = cst.tile([KD, 1], F32, tag="bq_c")
                nc.sync.dma_start(bq_c[:], bqc_d[:])
                bk_c = cst.tile([KD, 1], F32, tag="bk_c")
                nc.sync.dma_start(bk_c[:], bkc_d[:])
                bv_r = cst.tile([1, KD], F32, tag="bv_r")
                nc.sync.dma_start(bv_r[:], bvr_d[:])
                ident = cst.tile([128, 128], F32, tag="ident")
                nc.sync.dma_start(ident[:], ident_d.ap())
                ones_row = cst.tile([1, 128], F32, tag="ones_row")
                nc.sync.dma_start(ones_row[:], ones32_d[:])
                ones_col = cst.tile([128, 1], F32R, tag="ones_col")
                nc.sync.dma_start(ones_col[:], onescol_d[:])
                mshift_col = cst.tile([128, 1], F32, tag="mshift_col")
                nc.vector.memset(mshift_col[:], -M_SHIFT)

                # augmented projections: row KD holds ones (K side) / -c (Q side)
                kT_aug = cst.tile([KD + 1, S], F32R, tag="kT_aug")
                qT_aug = cst.tile([KD + 1, HALF], F32R, tag="qT_aug")
                nc.sync.dma_start(kT_aug[KD:KD + 1, :], ones_d[:])

                v_sb = [vpool.tile([128, KD], F32R, name=f"v{jc}", tag=f"v{jc}")
                        for jc in range(NJC)]

                # ---- projections (PE, fp32r) ------------------------------------
                if phase < 2:
                    for ic in range(NCHUNK):
                        o_sb = work.tile([128, KD], F32, tag="o_sb")
                        nc.vector.memset(o_sb[:], 0.0)
                        nc.sync.dma_start(out_d[ic * 128:(ic + 1) * 128, :], o_sb[:])
                    continue
                with tc.tile_pool(name="pp", bufs=2, space="PSUM") as pp:
                    # Q^T [KD, HALF] from the first HALF columns of xT (own rows)
                    for it in range(HALF // 512):
                        sl = slice(it * 512, (it + 1) * 512)
                        q_ps = pp.tile([KD, 512], F32, tag="q_ps")
                        nc.tensor.matmul(q_ps[:], wq[0][:], xt[0][:, sl],
                                         start=True, stop=False)
                        nc.tensor.matmul(q_ps[:], wq[1][:], xt[1][:, sl],
                                         start=False, stop=True)
                        # bias add serves as PSUM->SBUF eviction
                        nc.vector.tensor_scalar(qT_aug[0:KD, sl], q_ps[:], bq_c[:],
                                                None, ADD)
                    for it in range(0 if phase == 22 else S // 512):
                        sl = slice(it * 512, (it + 1) * 512)
                        k_ps = pp.tile([KD, 512], F32, tag="k_ps")
                        nc.tensor.matmul(k_ps[:], wk[0][:], xt[0][:, sl],
                                         start=True, stop=False)
                        nc.tensor.matmul(k_ps[:], wk[1][:], xt[1][:, sl],
                                         start=False, stop=True)
                        nc.vector.tensor_scalar(kT_aug[0:KD, sl], k_ps[:], bk_c[:],
                                                None, ADD)
                    # V natural [S, KD] per j-chunk; bias via rank-1 accumulate
                    for jc in range(NJC):
                        sl = slice(jc * 128, (jc + 1) * 128)
                        v_ps = pp.tile([128, KD], F32, tag="v_ps")
                        nc.tensor.matmul(v_ps[:], xt[0][:, sl], wv[0][:],
                                         start=True, stop=False)
                        nc.tensor.matmul(v_ps[:], xt[1][:, sl], wv[1][:],
                                         start=False, stop=False)
                        nc.tensor.matmul(v_ps[:], ones_row[:], bv_r[:],
                                         start=False, stop=True)
                        nc.vector.tensor_copy(v_sb[jc][:], v_ps[:])

                if phase < 3 or phase in (21, 22):
                    for ic in range(NCHUNK):
                        o_sb = work.tile([128, KD], F32, tag="o_sb")
                        nc.vector.memset(o_sb[:], 0.0)
                        nc.sync.dma_start(out_d[ic * 128:(ic + 1) * 128, :], o_sb[:])
                    continue
                # ---- round 1: softmax denominators ------------------------------
                # den_parts columns: [jh=0 chunks 0..15 | jh=1 chunks 16..31]
                den_parts = cst.tile([128, 2 * NCHUNK], F32, tag="den_parts")
                with tc.tile_pool(name="s1", bufs=2, space="PSUM") as s1p:
                    for ic in range(NCHUNK):
                        isl = slice(ic * 128, (ic + 1) * 128)
                        for jh in range(2):
                            s1_ps = s1p.tile([128, HALF], F32, tag="s1")
                            for it in range(4):
                                jsl = slice(jh * HALF + it * 512,
                                            jh * HALF + (it + 1) * 512)
                                nc.tensor.matmul(s1_ps[:, it * 512:(it + 1) * 512],
                                                 qT_aug[0:KD, isl],
                                                 kT_aug[0:KD, jsl],
                                                 start=True, stop=True)
                            e_scr = scr.tile([128, HALF], F32, tag="e_scr")
                            nc.scalar.activation(
                                e_scr[:], s1_ps[:], EXP, bias=mshift_col[:],
                                accum_out=den_parts[:, jh * NCHUNK + ic:
                                                    jh * NCHUNK + ic + 1])

                # ---- thresholds: -c = lnS - ln(den) -----------------------------
                den_all = cst.tile([128, NCHUNK], F32, tag="den_all")
                nc.vector.tensor_tensor(den_all[:], den_parts[:, 0:NCHUNK],
                                        den_parts[:, NCHUNK:2 * NCHUNK], ADD)
                ln_den = cst.tile([128, NCHUNK], F32, tag="ln_den")
                nc.scalar.activation(ln_den[:], den_all[:], LN)
                # den_true = den' * e^M  =>  -c = lnS - M - ln(den')
                neg_c = cst.tile([128, NCHUNK], F32, tag="neg_c")
                nc.vector.tensor_scalar(neg_c[:], ln_den[:], LN_S - M_SHIFT, -1.0,
                                        SUB, MULT)
                with tc.tile_pool(name="cps", bufs=2, space="PSUM") as cps:
                    for ic in range(NCHUNK):
                        ncT_ps = cps.tile([1, 128], F32, tag="ncT")
                        nc.tensor.transpose(ncT_ps[:], neg_c[:, ic:ic + 1],
                                            ident[:])
                        nc.vector.tensor_copy(
                            qT_aug[KD:KD + 1, ic * 128:(ic + 1) * 128],
                            ncT_ps[:])

                if phase < 4:
                    for ic in range(NCHUNK):
                        o_sb = work.tile([128, KD], F32, tag="o_sb")
                        nc.vector.memset(o_sb[:], 0.0)
                        nc.sync.dma_start(out_d[ic * 128:(ic + 1) * 128, :], o_sb[:])
                    continue
                # ---- bf16 hi/lo split of score operands (3-matmul exact-ish) ----
                BF16 = mybir.dt.bfloat16
                kh = cst.tile([KD + 1, S], BF16, tag="kh")
                nc.vector.tensor_copy(kh[:], kT_aug[:])
                kl = cst.tile([KD + 1, S], BF16, tag="kl")
                nc.vector.tensor_tensor(kl[:], kT_aug[:], kh[:], SUB)
                qh = cst.tile([KD + 1, HALF], BF16, tag="qh")
                nc.vector.tensor_copy(qh[:], qT_aug[:])
                ql = cst.tile([KD + 1, HALF], BF16, tag="ql")
                nc.vector.tensor_tensor(ql[:], qT_aug[:], qh[:], SUB)

                # ---- round 2: masked, pre-normalized scores + V matmul ----------
                outT_sb = cst.tile([KD, HALF], F32, tag="outT_sb")
                rho_sb = cst.tile([1, HALF], F32, tag="rho_sb")
                with (
                    tc.tile_pool(name="u", bufs=2, space="PSUM") as up,
                    tc.tile_pool(name="ot", bufs=1, space="PSUM") as otp,
                    tc.tile_pool(name="rho", bufs=1, space="PSUM") as rhop,
                ):
                    for h in range(2):
                        hsl = slice(h * 1024, (h + 1) * 1024)
                        oT_ps = otp.tile([KD, 1024], F32, tag="oT")
                        rho_ps = rhop.tile([1, 1024], F32, tag="rho")
                        for jc in range(NJC):
                            jsl = slice(jc * 128, (jc + 1) * 128)
                            qsl = slice(h * 1024, (h + 1) * 1024)
                            u_ps = up.tile([128, 1024], F32, tag="u")
                            for it in range(2):
                                osl = slice(it * 512, (it + 1) * 512)
                                ssl = slice(h * 1024 + it * 512,
                                            h * 1024 + (it + 1) * 512)
                                nc.tensor.matmul(u_ps[:, osl], kh[:, jsl],
                                                 qh[:, ssl], start=True, stop=False)
                                nc.tensor.matmul(u_ps[:, osl], kh[:, jsl],
                                                 ql[:, ssl], start=False, stop=False)
                                nc.tensor.matmul(u_ps[:, osl], kl[:, jsl],
                                                 qh[:, ssl], start=False, stop=True)
                            eu = work.tile([128, 1024], F32R, tag="eu")
                            nc.scalar.activation(eu[:], u_ps[:], EXP)
                            for it in range(2):
                                osl = slice(it * 512, (it + 1) * 512)
                                nc.tensor.matmul(rho_ps[:, osl], ones_col[:],
                                                 eu[:, osl],
                                                 start=(jc == 0),
                                                 stop=(jc == NJC - 1))
                            g = work.tile([128, 1024], F32R, tag="g")
                            nc.vector.scalar_tensor_tensor(
                                g[:], eu[:], 1.0, eu[:], IS_GT, MULT)
                            for it in range(2):
                                nc.tensor.matmul(
                                    oT_ps[:, it * 512:(it + 1) * 512],
                                    v_sb[jc][:],
                                    g[:, it * 512:(it + 1) * 512],
                                    start=(jc == 0), stop=(jc == NJC - 1))
                        nc.vector.tensor_copy(outT_sb[:, hsl], oT_ps[:])
                        nc.vector.tensor_copy(rho_sb[0:1, hsl], rho_ps[:])

                # ---- transpose result to [i, KD], scale by 1/rho, store ---------
                rho_col = cst.tile([128, NCHUNK], F32, tag="rho_col")
                inv_rho = cst.tile([128, NCHUNK], F32, tag="inv_rho")
                with tc.tile_pool(name="tr", bufs=2, space="PSUM") as trp:
                    for ic in range(NCHUNK):
                        rc_ps = trp.tile([128, 1], F32, tag="rc")
                        nc.tensor.transpose(
                            rc_ps[:], rho_sb[0:1, ic * 128:(ic + 1) * 128],
                            ident[0:1, 0:1])
                        nc.vector.tensor_copy(rho_col[:, ic:ic + 1], rc_ps[:])
                    nc.vector.reciprocal(inv_rho[:], rho_col[:])
                    for ic in range(NCHUNK):
                        isl = slice(ic * 128, (ic + 1) * 128)
                        tr_ps = trp.tile([128, KD], F32, tag="tr")
                        nc.tensor.transpose(tr_ps[:], outT_sb[:, isl],
                                            ident[0:KD, 0:KD])
                        o_sb = work.tile([128, KD], F32, tag="o_sb")
                        nc.vector.tensor_scalar(o_sb[:], tr_ps[:],
                                                inv_rho[:, ic:ic + 1], None,
                                                MULT)
                        nc.sync.dma_start(out_d[isl, :], o_sb[:])

    nc.compile()
    return nc


# ---------------------------------------------------------------------------
# Host side: shard, run on 8 cores, gather.
# ---------------------------------------------------------------------------

_CACHE: dict = {}


def _in_maps(x, Wq, bq, Wk, bk, Wv, bv):
    maps = []
    for c in range(NCORES):
        b, h = divmod(c, 2)
        xb = np.asarray(x[b], dtype=np.float32)
        # rotate rows so this core's query rows come first, then transpose
        xrot = np.roll(xb, -h * HALF, axis=0)
        maps.append({
            "xT": np.ascontiguousarray(xrot.T),
            "Wq": np.ascontiguousarray(Wq, dtype=np.float32),
            "Wk": np.ascontiguousarray(Wk, dtype=np.float32),
            "Wv": np.ascontiguousarray(Wv, dtype=np.float32),
            "bq_col": np.ascontiguousarray(np.asarray(bq, np.float32).reshape(KD, 1)),
            "bk_col": np.ascontiguousarray(np.asarray(bk, np.float32).reshape(KD, 1)),
            "bv_row": np.ascontiguousarray(
                np.asarray(bv, np.float32).reshape(1, KD)),
            "ones": np.ones((1, S), dtype=np.float32),
            "ones32": np.ones((1, 128), dtype=np.float32),
            "ones_col": np.ones((128, 1), dtype=np.float32),
        })
    return maps


def get_runner():
    """Build the program once and return (nc, run_fn).

    run_fn(in_maps) -> list of per-core output dicts.  The jitted PJRT
    callable is cached so repeated kernel() calls don't recompile.
    """
    if "runner" in _CACHE:
        return _CACHE["runner"]

    nc = build_program()

    import jax
    from jax.sharding import Mesh, PartitionSpec
    from jax.experimental.shard_map import shard_map
    from concourse import bass2jax
    from concourse import mybir as _mybir

    bass2jax.install_neuronx_cc_hook()

    partition_name = nc.partition_id_tensor.name if nc.partition_id_tensor else None
    in_names, out_names, out_avals = [], [], []
    for alloc in nc.m.functions[0].allocations:
        if not isinstance(alloc, _mybir.MemoryLocationSet):
            continue
        name = alloc.memorylocations[0].name
        if alloc.kind == "ExternalInput":
            if name != partition_name:
                in_names.append(name)
        elif alloc.kind == "ExternalOutput":
            out_names.append(name)
            out_avals.append(jax.core.ShapedArray(
                tuple(alloc.tensor_shape), _mybir.dt.np(alloc.dtype)))
    n_params = len(in_names)
    all_names = in_names + out_names
    if partition_name is not None:
        all_names = all_names + [partition_name]

    def _body(*args):
        operands = list(args)
        if partition_name is not None:
            operands.append(bass2jax.partition_id_tensor())
        outs = bass2jax._bass_exec_p.bind(
            *operands,
            out_avals=tuple(out_avals),
            in_names=tuple(all_names),
            out_names=tuple(out_names),
            lowering_input_output_aliases=(),
            sim_require_finite=False,
            sim_require_nnan=False,
            nc=nc,
        )
        return tuple(outs)

    # Bust any HLO-module-level executable caching when the program changes:
    # the jit module name includes a content hash of the BIR.
    import hashlib
    _body.__name__ = "body_" + hashlib.sha256(nc.to_json_bytes()).hexdigest()[:12]

    devices = jax.devices()[:NCORES]
    mesh = Mesh(np.asarray(devices), ("core",))
    n_outs = len(out_names)
    sharded = jax.jit(shard_map(
        _body, mesh=mesh,
        in_specs=(PartitionSpec("core"),) * (n_params + n_outs),
        out_specs=(PartitionSpec("core"),) * n_outs,
        check_rep=False,
    ), keep_unused=True)

    def run_fn(maps):
        concat_in = [
            np.concatenate([np.asarray(maps[c][nm]) for c in range(NCORES)], axis=0)
            for nm in in_names
        ]
        concat_zero = [
            np.zeros((NCORES * av.shape[0], *av.shape[1:]), av.dtype)
            for av in out_avals
        ]
        outs = sharded(*concat_in, *concat_zero)
        return [
            {nm: np.asarray(outs[i]).reshape(NCORES, *out_avals[i].shape)[c]
             for i, nm in enumerate(out_names)}
            for c in range(NCORES)
        ]

    _CACHE["runner"] = (nc, run_fn, sharded, in_names, out_avals, out_names)
    return _CACHE["runner"]


def kernel(x, Wq, bq, Wk, bk, Wv, bv):
    _, run_fn, *_ = get_runner()
    results = run_fn(_in_maps(x, Wq, bq, Wk, bk, Wv, bv))
    out = np.empty((B, S, KD), dtype=np.float32)
    for c in range(NCORES):
        b, h = divmod(c, 2)
        out[b, h * HALF:(h + 1) * HALF, :] = results[c]["out"]
    return out

